# revision 51
# baseline (speedup 1.0000x reference)
"""ABCAttention (fla chunk_abc semantics) on 8 TRN2 NeuronCores.

Sharding: core c -> (batch b = c//2, head-pair hg = c%2). Each core computes
its batch's projections for its 2 heads (512 of 1024 feature columns), runs
the chunked ABC scan for those 2 (b,h) pairs, applies the gated-RMSNorm
epilogue, and produces a partial o_w product (its 512 rows of o_w). The host
sums the two partials per batch and transposes (device emits out^T).

Math: the ABC scan is rewritten in chunked form (C=128) with UNNORMALIZED
slot state:
    W_t[m]  = cumsum_t exp(s)          (running log-normalizer; z = ln W)
    ok_t    = (q_t @ Uk + A_masked @ E) / W_t,  Uk = sum_j k_j E_j^T
    qw_t    = softmax_m(ok_t) / W_t
    ov_t    = qw_t @ Uv + (E @ qw^T)^T_masked @ V,  Uv = sum_j E_j v_j^T

Perf notes (vs the first working version):
  - the epilogue + out-projection are pipelined ONE SLAB LATE: emitted
    after the next slab's s/v/g projections, so the PE never waits on
    og^T transposes and the epilogue's DVE burst spreads over the next
    slab's projection phase instead of colliding with rope at slab
    boundaries;
  - k^T / og^T transposes ride the SP queue, E^T transposes the ACT
    queue (gated only on ACT's own just-produced exp, so no inversion);
    qw^T stays on the PE (DMA-queue latency is too slow for the
    softmax-chain critical path);
  - engine queues stay phase-dedicated: mid-kernel stores only on the
    gpsimd SWDGE queue; late-gated DMAs must never sit in front of
    early-phase work on an in-order queue (tried once: +95us);
  - x loads for slabs>=1 are ONE descriptor-rich DMA each (engine-side
    dma_start costs ~0.6us; 1 instead of 8), issued a full slab early;
  - psAt PSUM pool is double-buffered so head h1's At/ok matmuls start
    while h0's softmax chain drains (psP shrinks to 2 bufs to fit);
  - trig is loaded in bf16 and wq/wk loads precede it, easing the
    saturated ~30us startup DMA window;
  - psum->sbuf copies balanced across ACT (E exp, V, ovc, u1, osb-even)
    and DVE (G, qw^T, osb-odd) so neither in-order queue convoys the
    scan chain; output staging/stores bf16 (host sums partials in f32).

NOTE do not write any tile in-place / alias pool tags across a
producer-consumer pair (e.g. out=in0 tensor_tensor or bufs=1 chains):
the tile dependency tracker either deadlocks or corrupts. A batched
16-block og^T DMA transpose also corrupts (4-block transposes work).
"""
import sys
import numpy as np
import ml_dtypes

sys.path.insert(0, '/opt/trn_rl_repo')

import concourse.bass as bass        # noqa: E402
import concourse.bacc as bacc        # noqa: E402
import concourse.mybir as mybir      # noqa: E402
import concourse.tile as tile        # noqa: E402

f32 = mybir.dt.float32
f32r = mybir.dt.float32r
bf16 = mybir.dt.bfloat16
AF = mybir.ActivationFunctionType
ALU = mybir.AluOpType
AX = mybir.AxisListType

B, T, D, H = 4, 2048, 1024, 4
DK = DV = M = 256
HALF = 128
GATE_NORM = 16.0
EPS = 1e-5
ROPE_BASE = 10000.0
QSCALE = 1.0 / 16.0          # DK ** -0.5

NCORE = 8
COLS = 512                   # feature columns per core (2 heads)
SLAB = 512                   # tokens per pipeline slab
NSLAB = T // SLAB
C = 128                      # scan chunk length
CPS = SLAB // C              # chunks per slab
KB = D // 128                # 8 contraction blocks


def ts(i, n=128):
    return bass.ts(i, n)


def _patch_act_tables():
    """Keep only natural_log_exp_and_others selectable (ids preserved) so the
    table-load pass stops thrashing exp_and_others <-> natural_log."""
    if getattr(bacc, "_abc_act_patch", False):
        return
    import concourse.hw_specs as hw_specs
    orig = hw_specs.get_activation_tables

    def patched(module_arch):
        tabs = orig(module_arch)
        keep = "natural_log_exp_and_others"
        return {k: (v if k == keep else set()) for k, v in tabs.items()}

    bacc.get_activation_tables = patched
    bacc._abc_act_patch = True


def build():
    _patch_act_tables()
    nc = bacc.Bacc(None, target_bir_lowering=False)

    xT_e = nc.declare_dram_parameter("xT", [D, T], bf16, isOutput=False)
    w_e = {}
    for nm in ("wq", "wk", "ws", "wv", "wg"):
        w_e[nm] = nc.declare_dram_parameter(nm, [D, COLS], bf16, isOutput=False)
    wsg_e = nc.declare_dram_parameter("wsg", [D, 2], bf16, isOutput=False)
    wo_e = nc.declare_dram_parameter("wo", [COLS, D], bf16, isOutput=False)
    cos_e = nc.declare_dram_parameter("cosT", [HALF, T], bf16, isOutput=False)
    sin_e = nc.declare_dram_parameter("sinT", [HALF, T], bf16, isOutput=False)
    trq_e = nc.declare_dram_parameter("trilq", [128, 128], f32, isOutput=False)
    tr1_e = nc.declare_dram_parameter("tril1", [128, 128], f32, isOutput=False)
    tr1b_e = nc.declare_dram_parameter("tril1b", [128, 128], bf16, isOutput=False)
    idb_e = nc.declare_dram_parameter("identb", [128, 128], bf16, isOutput=False)
    one_e = nc.declare_dram_parameter("onesc", [1, 128], bf16, isOutput=False)
    onk_e = nc.declare_dram_parameter("onek", [128, 1], bf16, isOutput=False)
    zr_e = nc.declare_dram_parameter("zeros", [128, 512], bf16, isOutput=False)
    out_e = nc.declare_dram_parameter("outT", [D, T], bf16, isOutput=True)

    with tile.TileContext(nc) as tc:
        with (tc.tile_pool(name="weights", bufs=1) as wp,
              tc.tile_pool(name="consts", bufs=1) as cp,
              tc.tile_pool(name="slab", bufs=2) as sp,
              tc.tile_pool(name="scan", bufs=2) as kp,
              tc.tile_pool(name="psP", bufs=2, space="PSUM") as pp,
              tc.tile_pool(name="psAt", bufs=2, space="PSUM") as pat,
              tc.tile_pool(name="psOk", bufs=2, space="PSUM") as pok,
              tc.tile_pool(name="psSt", bufs=2, space="PSUM") as pst):
            # ---- resident constants & weights -------------------------------
            wt = {}
            for nm in ("wq", "wk", "ws", "wv", "wg"):
                wt[nm] = [wp.tile([128, COLS], bf16, tag=f"{nm}{kb}", name=f"{nm}{kb}")
                          for kb in range(KB)]
            wsg_t = [wp.tile([128, 2], bf16, tag=f"wsg{kb}", name=f"wsg{kb}") for kb in range(KB)]
            wo_t = [wp.tile([128, D], bf16, tag=f"wo{q}", name=f"wo{q}") for q in range(4)]
            # critical-path weights first (sg chain + q/k projections);
            # spread issue across engine queues to parallelize descriptor setup
            engs = [nc.sync, nc.scalar, nc.gpsimd]
            for kb in range(KB):
                engs[kb % 3].dma_start(wsg_t[kb][:], wsg_e[ts(kb), :])
            def _weight_dmas_qk():
                i = 0
                for nm in ("wq", "wk"):
                    for kb in range(KB):
                        engs[i % 3].dma_start(wt[nm][kb][:], w_e[nm][ts(kb), :])
                        i += 1

            def _weight_dmas_rest():
                i = 0
                for nm in ("ws", "wv", "wg"):
                    for kb in range(KB):
                        engs[i % 3].dma_start(wt[nm][kb][:], w_e[nm][ts(kb), :])
                        i += 1
                for q in range(4):
                    engs[q % 3].dma_start(wo_t[q][:], wo_e[ts(q), :])
            trilq = cp.tile([128, 128], f32, tag="trilq", name="trilq")
            tril1 = cp.tile([128, 128], f32, tag="tril1", name="tril1")
            tril1r = cp.tile([128, 128], bf16, tag="tril1r", name="tril1r")
            identr = cp.tile([128, 128], bf16, tag="identr", name="identr")
            onescr = cp.tile([1, 128], bf16, tag="onescr", name="onescr")
            onekr = cp.tile([128, 1], bf16, tag="onekr", name="onekr")
            zeros2 = cp.tile([2, SLAB], f32, tag="zeros2", name="zeros2")
            nc.vector.memset(zeros2[:], 0.0)

            # ---- persistent scan state (ping-pong SBUF tiles) ---------------
            uk_cur, uv_cur = [], []
            for h in range(2):
                uk = kp.tile([128, 2 * M], bf16, tag=f"uk{h}", name=f"uk{h}", bufs=2)
                uv = kp.tile([128, 2 * DV], bf16, tag=f"uv{h}", name=f"uv{h}", bufs=2)
                uk_cur.append(uk)
                uv_cur.append(uv)

            def _const_dmas():
                # issued after the weight/x loads: only needed by the scan
                nc.sync.dma_start(trilq[:], trq_e[:])
                nc.sync.dma_start(tril1[:], tr1_e[:])
                nc.sync.dma_start(tril1r[:], tr1b_e[:])
                nc.sync.dma_start(identr[:], idb_e[:])
                nc.sync.dma_start(onescr[:], one_e[:])
                nc.sync.dma_start(onekr[:], onk_e[:])
                for h in range(2):
                    nc.sync.dma_start(uk_cur[h][:], zr_e[:])
                    nc.sync.dma_start(uv_cur[h][:], zr_e[:])
            wlastf = kp.tile([1, COLS], f32, tag="wlastf", name="wlastf", bufs=2)
            nc.vector.memset(wlastf[:], 0.0)
            wlast = kp.tile([1, COLS], bf16, tag="wlastb", name="wlastb", bufs=5)
            nc.vector.memset(wlast[:], 0.0)

            sg_carry = kp.tile([2, 1], f32, tag="sgc", name="sgc", bufs=2)
            nc.vector.memset(sg_carry[:], 0.0)

            def load_slab(s, spread=False):
                tok = slice(s * SLAB, (s + 1) * SLAB)
                big = sp.tile([128, KB * SLAB], bf16, tag="xsb", name="xsb",
                              bufs=2)
                xs = [big[:, kb * SLAB:(kb + 1) * SLAB] for kb in range(KB)]
                if spread:
                    engs2 = [nc.sync, nc.scalar, nc.gpsimd]
                    for kb in range(KB):
                        engs2[kb % 3].dma_start(xs[kb], xT_e[ts(kb), tok])
                else:
                    # one descriptor-rich DMA (engine-side issue cost ~0.6us
                    # each; 1 instead of 8)
                    nc.gpsimd.dma_start(
                        big[:].rearrange("p (kb t) -> p kb t", t=SLAB),
                        xT_e.rearrange("(kb p) t -> p kb t", p=128)[:, :, tok])
                return xs

            def sg_chain(xs):
                nonlocal sg_carry
                ps_sg = pst.tile([2, SLAB], f32, tag="state", name="ps_sg")
                for kb in range(KB):
                    nc.tensor.matmul(ps_sg[:], wsg_t[kb][:], xs[kb][:],
                                     start=(kb == 0), stop=(kb == KB - 1))
                e_sg = kp.tile([2, SLAB], f32, tag="sgtmp", name="esg", bufs=2)
                nc.scalar.activation(e_sg[:], ps_sg[:], AF.Exp, scale=-1.0)
                u_sg = kp.tile([2, SLAB], f32, tag="sgtmp", name="usg", bufs=2)
                nc.vector.tensor_scalar_add(u_sg[:], e_sg[:], 1.0)
                l_sg = kp.tile([2, SLAB], f32, tag="sgtmp", name="lsg", bufs=2)
                nc.scalar.activation(l_sg[:], u_sg[:], AF.Ln)
                cum = kp.tile([2, SLAB], f32, tag="sgtmp", name="cum", bufs=2)
                nc.vector.tensor_tensor_scan(cum[:], l_sg[:], zeros2[:],
                                             sg_carry[:], ALU.add, ALU.add)
                sg_carry = kp.tile([2, 1], f32, tag="sgc", name="sgc", bufs=2)
                nc.scalar.copy(sg_carry[:], cum[:, SLAB - 1:SLAB])
                lam = kp.tile([2, SLAB], bf16, tag="lam", name="lam", bufs=1)
                nc.scalar.activation(lam[:], cum[:], AF.Exp,
                                     scale=-1.0 / GATE_NORM)
                lam1 = kp.tile([1, SLAB], bf16, tag="lam1", name="lam1", bufs=1)
                nc.gpsimd.dma_start(lam1[:], lam[1:2, :])
                lam_bc = []
                for h in range(2):
                    bcst = kp.tile([128, SLAB], bf16, tag=f"lamb{h}", name=f"lamb{h}")
                    nc.gpsimd.partition_broadcast(
                        bcst[:], lam[0:1, :] if h == 0 else lam1[:])
                    lam_bc.append(bcst)
                return lam_bc

            def load_trig(s, spread=False):
                tok2 = slice(s * SLAB, (s + 1) * SLAB)
                c_t = sp.tile([HALF, SLAB], bf16, tag="cos_sl", name="cos_sl", bufs=2)
                s_t = sp.tile([HALF, SLAB], bf16, tag="sin_sl", name="sin_sl", bufs=2)
                eng = nc.sync if spread else nc.gpsimd
                eng.dma_start(c_t[:], cos_e[:, tok2])
                eng.dma_start(s_t[:], sin_e[:, tok2])
                return c_t, s_t

            def t3(ap, t=128):
                # view [128, N] as [128, N//t, t] for batched DMA transpose
                return ap.rearrange("p (j t) -> p j t", t=t)

            def emit_epilogue(st):
                # rsqrt cols, swish gate, og, og^T, out proj, stores — for
                # the PREVIOUS slab (pipelined one slab late)
                ssq, rden2_all = st["ssq"], st["rden2"]
                ov_sb, G_t, rden_t = st["ov"], st["G"], st["rden"]
                tok, last = st["tok"], st["last"]
                sqn = kp.tile([128, 2 * CPS], f32, tag="sqn", name="sqn")
                nc.vector.tensor_tensor(sqn[:], ssq[:], rden2_all[:], ALU.mult)
                vv = kp.tile([128, 2 * CPS], f32, tag="vv", name="vv")
                nc.vector.tensor_scalar(vv[:], sqn[:], 1.0 / DV, EPS,
                                        ALU.mult, ALU.add)
                lnv = kp.tile([128, 2 * CPS], f32, tag="lnv", name="lnv")
                nc.scalar.activation(lnv[:], vv[:], AF.Ln)
                rsq = kp.tile([128, 2 * CPS], f32, tag="rsq", name="rsq")
                nc.scalar.activation(rsq[:], lnv[:], AF.Exp, scale=-0.5)

                # og^T gathered per 128-feature block: ogt[:, q*SLAB+t]
                ogt = sp.tile([128, 4 * SLAB], bf16, tag="ogt", name="ogt")
                for ci in range(CPS):
                    gch = G_t[ci]
                    eneg = kp.tile([128, COLS], bf16, tag="eneg", name="eneg")
                    nc.scalar.activation(eneg[:], gch[:], AF.Exp, scale=-1.0)
                    u1 = kp.tile([128, COLS], f32, tag="u1", name="u1", bufs=1)
                    nc.scalar.activation(u1[:], eneg[:], AF.Copy, bias=1.0)
                    sig = kp.tile([128, COLS], f32, tag="sig", name="sig", bufs=1)
                    nc.vector.reciprocal_approx_fast(sig[:], u1[:])
                    p1 = kp.tile([128, COLS], bf16, tag="p1", name="p1")
                    # SBUF-only operands -> legal on Pool; frees DVE, and the
                    # epilogue has a full slab of slack
                    nc.gpsimd.tensor_tensor(p1[:], ov_sb[ci][:], gch[:],
                                            ALU.mult)
                    og = kp.tile([128, COLS], bf16, tag="og", name="og")
                    for h in range(2):
                        hsl = slice(h * M, (h + 1) * M)
                        rr = kp.tile([128, 1], f32, tag="rr", name="rr")
                        nc.vector.tensor_tensor(
                            rr[:], rsq[:, 2 * ci + h:2 * ci + h + 1],
                            rden_t[(ci, h)][:], ALU.mult)
                        nc.vector.scalar_tensor_tensor(
                            og[:, hsl], p1[:, hsl], rr[:], sig[:, hsl],
                            ALU.mult, ALU.mult)
                    nc.sync.dma_start(
                        ogt[:].rearrange("p (j t) -> p j t", t=SLAB)[
                            :, :, ci * 128:(ci + 1) * 128],
                        og[:], transpose=True)
                # stores: gpsimd-only mid-kernel (keeps sync/scalar queues
                # free of late-gated waits); spread across 3 queues for the
                # final slab where nothing follows them
                st_engs = ([nc.gpsimd] if not last
                           else [nc.gpsimd, nc.sync, nc.scalar])
                cp_engs = [nc.scalar.copy, nc.vector.tensor_copy]
                for ct in range(8):
                    ps_o = pst.tile([128, SLAB], f32, tag="state", name="ps_o")
                    for q in range(4):
                        nc.tensor.matmul(ps_o[:], wo_t[q][:, ts(ct)],
                                         ogt[:, q * SLAB:(q + 1) * SLAB],
                                         start=(q == 0), stop=(q == 3))
                    osb = kp.tile([128, SLAB], bf16, tag="osb", name="osb",
                                  bufs=4)
                    cp_engs[ct % 2](osb[:], ps_o[:])
                    st_engs[ct % len(st_engs)].dma_start(out_e[ts(ct), tok],
                                                         osb[:])

            prev_state = None
            xs = load_slab(0, spread=True)
            _weight_dmas_qk()
            trig = load_trig(0, spread=True)
            _weight_dmas_rest()
            _const_dmas()
            lam_bc = sg_chain(xs)
            xs_next = load_slab(1)
            trig_next = load_trig(1)
            for s in range(NSLAB):
                tok = slice(s * SLAB, (s + 1) * SLAB)
                cos_sl, sin_sl = trig

                # ---- q/k projections (feature-major) + rope ----------------
                qT, kT = [], []           # 4 col-tiles each: [128, SLAB] bf16
                for nm, dest, gated in (("wq", qT, False), ("wk", kT, True)):
                    for h in range(2):
                        ps1 = pp.tile([128, SLAB], f32, tag="proj", name="ps1")
                        ps2 = pp.tile([128, SLAB], f32, tag="proj", name="ps2")
                        for kb in range(KB):
                            nc.tensor.matmul(
                                ps1[:], wt[nm][kb][:, ts(2 * h)], xs[kb][:],
                                start=(kb == 0), stop=(kb == KB - 1))
                        for kb in range(KB):
                            nc.tensor.matmul(
                                ps2[:], wt[nm][kb][:, ts(2 * h + 1)], xs[kb][:],
                                start=(kb == 0), stop=(kb == KB - 1))
                        cs = cos_sl[:]
                        sn = sin_sl[:]
                        m1 = kp.tile([128, SLAB], bf16, tag="ropeA", name="ropeA", bufs=1)
                        m2 = kp.tile([128, SLAB], bf16, tag="ropeB", name="ropeB", bufs=1)
                        o1 = kp.tile([128, SLAB], bf16, tag=f"{nm}o{2*h}", name=f"{nm}o{2*h}")
                        nc.vector.tensor_tensor(m1[:], ps1[:], cs, ALU.mult)
                        nc.vector.tensor_tensor(m2[:], ps2[:], sn, ALU.mult)
                        m3 = kp.tile([128, SLAB], bf16, tag="ropeC", name="ropeC", bufs=1)
                        m4 = kp.tile([128, SLAB], bf16, tag="ropeD", name="ropeD", bufs=1)
                        o2 = kp.tile([128, SLAB], bf16, tag=f"{nm}o{2*h+1}", name=f"{nm}o{2*h+1}")
                        nc.vector.tensor_tensor(m3[:], ps2[:], cs, ALU.mult)
                        nc.vector.tensor_tensor(m4[:], ps1[:], sn, ALU.mult)
                        if not gated:
                            nc.vector.tensor_tensor(o1[:], m1[:], m2[:],
                                                    ALU.subtract)
                            nc.vector.tensor_tensor(o2[:], m3[:], m4[:],
                                                    ALU.add)
                        else:
                            r1 = kp.tile([128, SLAB], bf16, tag="ropeE", name="ropeE", bufs=1)
                            r2 = kp.tile([128, SLAB], bf16, tag="ropeF", name="ropeF", bufs=1)
                            nc.vector.tensor_tensor(r1[:], m1[:], m2[:],
                                                    ALU.subtract)
                            nc.vector.tensor_tensor(r2[:], m3[:], m4[:],
                                                    ALU.add)
                            nc.vector.tensor_tensor(o1[:], r1[:], lam_bc[h][:],
                                                    ALU.mult)
                            nc.vector.tensor_tensor(o2[:], r2[:], lam_bc[h][:],
                                                    ALU.mult)
                        dest.extend([o1, o2])

                # k^T per dk-block: one DMA transpose covers all 4 chunks.
                # kt_all[i][:, ci*128:+128] = [tok, dk] for chunk ci.
                kt_all = []
                for i in range(4):
                    kt = sp.tile([128, SLAB], bf16, tag=f"kta{i}", name=f"kta{i}")
                    nc.sync.dma_start(t3(kt[:]), kT[i][:], transpose=True)
                    kt_all.append(kt)

                # ---- s/v/g projections (token-major) -----------------------
                E_t, V_t, G_t, em_t = [], [], [], []
                for ci in range(CPS):
                    for nm in ("ws", "wv", "wg"):
                        ps = pp.tile([128, COLS], f32, tag="proj", name="psvg")
                        for kb in range(KB):
                            nc.tensor.matmul(
                                ps[:, 0:COLS], xs[kb][:, ts(ci)], wt[nm][kb][:],
                                start=(kb == 0), stop=(kb == KB - 1))
                        if nm == "ws":
                            t = sp.tile([128, COLS], bf16, tag=f"E{ci}", name=f"E{ci}")
                            nc.scalar.activation(t[:], ps[:, 0:COLS], AF.Exp)
                            E_t.append(t)
                            # E^T: em[:, (2h+b)*128:+128] = [m, tok]
                            # on the scalar queue: gated only on ACT's own
                            # just-produced exp, and sync is transpose-bound
                            em = sp.tile([128, COLS], bf16, tag=f"em{ci}", name=f"em{ci}")
                            nc.scalar.dma_start(t3(em[:]), t[:], transpose=True)
                            em_t.append(em)
                        elif nm == "wv":
                            t = sp.tile([128, COLS], bf16, tag=f"V{ci}", name=f"V{ci}")
                            nc.scalar.copy(t[:], ps[:, 0:COLS])
                            V_t.append(t)
                        else:
                            t = sp.tile([128, COLS], bf16, tag=f"G{ci}", name=f"G{ci}")
                            nc.scalar.copy(t[:], ps[:, 0:COLS])
                            G_t.append(t)

                # ---- previous slab's epilogue + out projection (pipelined
                # one slab late: its PE work fills scan-phase gaps and its
                # DVE burst spreads over this slab's projection phase; the
                # PE never waits on og^T transposes) ------------------------
                if prev_state is not None:
                    emit_epilogue(prev_state)

                # ---- next-slab sg chain (loads were issued a slab early)
                if s + 1 < NSLAB:
                    lam_bc_next = sg_chain(xs_next)
                    if s + 2 < NSLAB:
                        xs_next2 = load_slab(s + 2)
                        trig_next2 = load_trig(s + 2)
                    else:
                        xs_next2, trig_next2 = None, None
                else:
                    lam_bc_next, xs_next2, trig_next2 = None, None, None

                # ---- W chain (chunk carries hoisted off the scan path) -----
                ssq = kp.tile([128, 2 * CPS], f32, tag="ssq", name="ssq")
                rden2_all = kp.tile([128, 2 * CPS], f32, tag="rden2a", name="rden2a")
                ov_sb = []
                wl_b = [wlast]
                for ci in range(CPS):
                    ps_wd = pat.tile([1, COLS], f32, tag="at", name="ps_wd")
                    nc.tensor.matmul(ps_wd[:], onekr[:], E_t[ci][:],
                                     start=True, stop=True)
                    wlast_new = kp.tile([1, COLS], f32, tag="wlastf", name="wlastf", bufs=2)
                    nc.vector.tensor_tensor(wlast_new[:], ps_wd[:],
                                            wlastf[:], ALU.add)
                    wlastf = wlast_new
                    if ci < CPS - 1:
                        wb = kp.tile([1, COLS], bf16, tag="wlastb", name="wlastb", bufs=5)
                        nc.scalar.copy(wb[:], wlast_new[:])
                        wl_b.append(wb)
                wlast = kp.tile([1, COLS], bf16, tag="wlastb", name="wlastb", bufs=5)
                nc.scalar.copy(wlast[:], wlastf[:])

                rW_t = []
                for ci in range(CPS):
                    ps_w = pp.tile([128, COLS], f32, tag="proj", name="ps_w")
                    nc.tensor.matmul(ps_w[:, 0:COLS], tril1r[:], E_t[ci][:],
                                     start=True, stop=False)
                    nc.tensor.matmul(ps_w[:, 0:COLS], onescr[:], wl_b[ci][:],
                                     start=False, stop=True)
                    rWf = kp.tile([128, COLS], f32, tag="rWf", name="rWf", bufs=1)
                    nc.vector.reciprocal_approx_fast(rWf[:], ps_w[:, 0:COLS])
                    rW = kp.tile([128, COLS], bf16, tag="rW", name="rW", bufs=3)
                    nc.vector.tensor_copy(rW[:], rWf[:])
                    rW_t.append(rW)

                # ---- per-chunk scan ----------------------------------------
                rden_t = {}
                for ci in range(CPS):
                    rW = rW_t[ci]

                    ovc = kp.tile([128, COLS], bf16, tag="ovc", name=f"ovc{ci}", bufs=9)
                    ov_sb.append(ovc)
                    qwc = kp.tile([128, COLS], bf16, tag="qwc", name="qwc", bufs=2)
                    qwt = kp.tile([128, COLS], bf16, tag="qwt", name="qwt", bufs=2)
                    for h in range(2):
                        hsl = slice(h * M, (h + 1) * M)
                        Ec = E_t[ci][:, hsl]
                        Vc = V_t[ci][:, hsl]
                        emc = em_t[ci]
                        qTc = [qT[2 * h][:, ts(ci)], qT[2 * h + 1][:, ts(ci)]]
                        kTc = [kT[2 * h][:, ts(ci)], kT[2 * h + 1][:, ts(ci)]]
                        ktc = [kt_all[2 * h][:, ts(ci)], kt_all[2 * h + 1][:, ts(ci)]]

                        # At = mask(K^T Q)/16   (shares a psum tile with St)
                        ps_at = pat.tile([128, 256], f32, tag="at", name="ps_at")
                        nc.tensor.matmul(ps_at[:, 0:128], kTc[0], qTc[0],
                                         start=True, stop=False)
                        nc.tensor.matmul(ps_at[:, 0:128], kTc[1], qTc[1],
                                         start=False, stop=True)
                        at_sb = kp.tile([128, 128], bf16, tag="at", name="at")
                        nc.vector.tensor_tensor(at_sb[:], ps_at[:, 0:128],
                                                trilq[:], ALU.mult)

                        # ok = At^T E + q Uk  (Uk pre-scaled by 1/16)
                        ps_ok = pok.tile([128, SLAB], f32, tag="okov", name="ps_ok")
                        ps_ov = ps_ok[:, 256:512]
                        nc.tensor.matmul(ps_ok[:, 0:M], at_sb[:], Ec,
                                         start=True, stop=False)
                        nc.tensor.matmul(ps_ok[:, 0:M], qTc[0],
                                         uk_cur[h][:, 0:M],
                                         start=False, stop=False)
                        nc.tensor.matmul(ps_ok[:, 0:M], qTc[1],
                                         uk_cur[h][:, M:2 * M],
                                         start=False, stop=True)
                        rWc = rW[:, hsl]
                        okn = kp.tile([128, M], f32, tag="okn", name="okn")
                        nc.vector.tensor_tensor(okn[:], ps_ok[:, 0:M], rWc,
                                                ALU.mult)
                        p = kp.tile([128, M], bf16, tag="p", name="p")
                        den = kp.tile([128, 1], f32, tag="den", name="den")
                        nc.scalar.activation(p[:], okn[:], AF.Exp,
                                             accum_out=den[:])
                        rden = kp.tile([128, 1], f32, tag=f"rden{ci}{h}",
                                       name="rden", bufs=2)
                        nc.vector.reciprocal_approx_fast(rden[:], den[:])
                        nc.vector.tensor_tensor(
                            rden2_all[:, 2 * ci + h:2 * ci + h + 1],
                            rden[:], rden[:], ALU.mult)
                        rden_t[(ci, h)] = rden
                        nc.vector.tensor_tensor(qwc[:, hsl], p[:], rWc, ALU.mult)

                        # qw^T for this head via PE transpose
                        ps_qt = pst.tile([128, 256], bf16, tag="state", name="ps_qt")
                        for blk in range(2):
                            nc.tensor.transpose(ps_qt[:, ts(blk)],
                                                qwc[:, 128 * (2 * h + blk):
                                                    128 * (2 * h + blk + 1)],
                                                identr[:])
                        nc.vector.tensor_copy(qwt[:, 256 * h:256 * (h + 1)],
                                              ps_qt[:])

                        # St = mask(E qw^T)
                        ps_st = ps_at[:, 128:256]
                        nc.tensor.matmul(ps_st, emc[:, ts(2 * h)],
                                         qwt[:, ts(2 * h)], start=True,
                                         stop=False)
                        nc.tensor.matmul(ps_st, emc[:, ts(2 * h + 1)],
                                         qwt[:, ts(2 * h + 1)], start=False,
                                         stop=True)
                        st_sb = kp.tile([128, 128], bf16, tag="st", name="st")
                        nc.vector.tensor_tensor(st_sb[:], ps_st,
                                                tril1[:], ALU.mult)

                        # ov = St^T V + qw Uv   (into ps_ok's 2nd half)
                        nc.tensor.matmul(ps_ov, st_sb[:], Vc,
                                         start=True, stop=False)
                        nc.tensor.matmul(ps_ov, qwt[:, ts(2 * h)],
                                         uv_cur[h][:, 0:DV],
                                         start=False, stop=False)
                        nc.tensor.matmul(ps_ov, qwt[:, ts(2 * h + 1)],
                                         uv_cur[h][:, DV:2 * DV],
                                         start=False, stop=True)
                        nc.scalar.copy(ovc[:, hsl], ps_ov)
                        sqs = kp.tile([128, M], bf16, tag="sqs", name="sqs", bufs=1)
                        nc.scalar.activation(
                            sqs[:], ps_ov, AF.Square,
                            accum_out=ssq[:, 2 * ci + h:2 * ci + h + 1])

                        # state updates
                        ps_dk = pst.tile([128, 2 * M], f32, tag="state", name="ps_dk")
                        nc.tensor.matmul(ps_dk[:, 0:M], ktc[0], Ec,
                                         start=True, stop=True)
                        nc.tensor.matmul(ps_dk[:, M:2 * M], ktc[1],
                                         Ec, start=True, stop=True)
                        uk_new = kp.tile([128, 2 * M], bf16, tag=f"uk{h}", name=f"uk{h}", bufs=2)
                        nc.vector.scalar_tensor_tensor(
                            uk_new[:], ps_dk[:], QSCALE, uk_cur[h][:],
                            ALU.mult, ALU.add)
                        uk_cur[h] = uk_new
                        ps_dv = pst.tile([128, 2 * DV], f32, tag="state", name="ps_dv")
                        nc.tensor.matmul(ps_dv[:, 0:DV], Ec[:, 0:128], Vc,
                                         start=True, stop=True)
                        nc.tensor.matmul(ps_dv[:, DV:2 * DV], Ec[:, 128:256],
                                         Vc, start=True, stop=True)
                        uv_new = kp.tile([128, 2 * DV], bf16, tag=f"uv{h}", name=f"uv{h}", bufs=2)
                        nc.vector.scalar_tensor_tensor(
                            uv_new[:], ps_dv[:], 1.0, uv_cur[h][:],
                            ALU.mult, ALU.add)
                        uv_cur[h] = uv_new

                # ---- capture epilogue state; emitted next iteration --------
                prev_state = {"ssq": ssq, "rden2": rden2_all, "ov": ov_sb,
                              "G": G_t, "rden": rden_t, "tok": tok,
                              "last": s == NSLAB - 1}
                xs = xs_next
                lam_bc = lam_bc_next
                trig = trig_next
                if s + 1 < NSLAB:
                    xs_next = xs_next2
                    trig_next = trig_next2
            emit_epilogue(prev_state)
    nc.compile()
    return nc


_CACHE = {}


def _host_inputs(hidden_states, q_w, k_w, v_w, g_w, s_w, sg_w, gn_w, o_w):
    half = HALF
    inv = 1.0 / (ROPE_BASE ** (np.arange(half, dtype=np.float64) / half))
    ang = np.arange(T, dtype=np.float64)[None, :] * inv[:, None]   # [half, T]
    cosT = np.cos(ang).astype(ml_dtypes.bfloat16)
    sinT = np.sin(ang).astype(ml_dtypes.bfloat16)
    jj, tt_ = np.meshgrid(np.arange(128), np.arange(128), indexing="ij")
    tril = (jj <= tt_).astype(np.float32)
    ones = np.ones((1, 128), np.float32)

    b16 = ml_dtypes.bfloat16
    in_maps = []
    for c in range(NCORE):
        b, hg = c // 2, c % 2
        cs = slice(hg * COLS, (hg + 1) * COLS)
        gn_ext = np.asarray(gn_w, np.float32)[np.arange(COLS) % DV]
        in_maps.append({
            "xT": np.ascontiguousarray(np.asarray(hidden_states[b], np.float32).T).astype(b16),
            "wq": np.ascontiguousarray(np.asarray(q_w, np.float32)[:, cs]).astype(b16),
            "wk": np.ascontiguousarray(np.asarray(k_w, np.float32)[:, cs]).astype(b16),
            "ws": np.ascontiguousarray(np.asarray(s_w, np.float32)[:, cs]).astype(b16),
            "wv": np.ascontiguousarray(np.asarray(v_w, np.float32)[:, cs]).astype(b16),
            "wg": np.ascontiguousarray(np.asarray(g_w, np.float32)[:, cs]).astype(b16),
            "wsg": np.ascontiguousarray(np.asarray(sg_w, np.float32)[:, 2 * hg:2 * hg + 2]).astype(b16),
            "wo": (np.asarray(o_w, np.float32)[cs, :]
                   * gn_ext[:, None]).astype(b16),
            "cosT": cosT, "sinT": sinT,
            "trilq": tril * np.float32(QSCALE), "tril1": tril,
            "tril1b": tril.astype(b16),
            "identb": np.eye(128, dtype=np.float32).astype(b16),
            "onesc": ones.astype(b16),
            "onek": np.ones((128, 1), b16),
            "zeros": np.zeros((128, 512), b16),
        })
    return in_maps


def kernel(**inputs):
    from concourse.bass_utils import run_bass_kernel_spmd
    if "nc" not in _CACHE:
        _CACHE["nc"] = build()
    nc = _CACHE["nc"]
    in_maps = _host_inputs(**inputs)
    r = run_bass_kernel_spmd(nc, in_maps, core_ids=list(range(NCORE)))
    out = np.empty((B, T, D), np.float32)
    for b in range(B):
        out[b] = (np.asarray(r.results[2 * b]["outT"], np.float32)
                  + np.asarray(r.results[2 * b + 1]["outT"], np.float32)).T
    return out

